# revision 9
# baseline (speedup 1.0000x reference)
"""Block-local sparse attention (LSG-style) on 8 TRN2 NeuronCores.

Sharding: the 32 (n, h) pairs are split 4-per-core (data/head parallel, no
collectives). Host-side numpy prep re-lays-out the inputs so the device
kernel needs no transposes, all bf16:

  - qt : Q^T per head [64, T]
  - lkt/skt/gkt: local/sparse/global K^T, token-padded with zeros
  - lv/sv/gv: V with a ones column appended (col 64), chunked [128, c, 65],
    and every row scaled by exp(mask): softmax(QK/8 + m) @ V is computed as
    sum_t exp(s_t) e^{m_t} [V_t, 1], then a divide by the accumulated last
    column — exact for any additive mask, and pad tokens (e^{m}=0) vanish
    from both numerator and denominator, so no mask row and no
    max-subtraction are needed (|QK|/8 is O(5), well within fp32 exp range).
    sv additionally holds 4 phase-shifted copies so the 32-token-granular
    sparse windows always start at partition 0.

The device processes query-block PAIRS: 9 score matmuls per pair (the
shared global chunk and the two shared local chunks stream both blocks'
256 q columns in one matmul) into a 3-bank PSUM region [128, 1536] laid
out so no matmul output crosses a bank, one wide exp(S/8) on ACT, then
12 PV matmuls (6 per block, N=65) accumulating [q, V|Z] and a
reciprocal-normalize on DVE.

Raw bass with hand-placed semaphores: this walrus build encodes at most one
sem wait per matmul/ACT instruction, which rules out TileContext (its
scheduler attaches several). The pipeline is software-pipelined by hand
(scores of pair p+2 issue right after PV of pair p so exp overlaps PE) with
double-buffered SBUF/PSUM and parity-split counting semaphores (a DMA batch
and its grandparent share a buffer; per-parity sems make "mine completed"
exact even if DMA queues complete out of order).
"""

from contextlib import ExitStack

import numpy as np

import concourse.bass as bass
import concourse.mybir as mybir
from concourse.bass_utils import run_bass_kernel_spmd

N, H, T, D = 2, 16, 4096, 64
B = 128          # query block
NB = T // B      # 32
G = 64           # global tokens
TSP = T // 4     # sparse tokens (1024)
NH = N * H       # 32
NCORES = 8
SL = NH // NCORES  # 4 heads per core
NP = SL * NB // 2  # 64 block-pairs per core
PPS = NB // 2      # 16 pairs per slot

LKT_W = T + 2 * B            # 4352 padded local tokens
SKT_W = TSP + 320            # 1344 padded sparse tokens
LV_C = LKT_W // 128          # 34 local V chunks
SV_C = 11                    # sparse V chunks per phase

F32 = mybir.dt.float32
BF16 = mybir.dt.bfloat16
GE = "sem-ge"

# column layout of the per-pair score/prob tile [128, 1536] (3 PSUM banks;
# regions never cross a 512-col bank boundary)
C_SP1A, C_SP1B = 0, 128
C_SP2A, C_SP2B = 256, 384
C_G = 512        # 256 wide: q of both blocks
C_LOC1 = 768     # 256 wide: local chunk b+1, both blocks
C_LOC0 = 1024    # 128: local chunk b, block A only
C_LOC2 = 1152    # 256 wide: local chunk b+2, both blocks
C_LOC3 = 1408    # 128: local chunk b+3, block B only


def _build_bass():
    nc = bass.Bass("TRN2", num_devices=NCORES, debug=False)

    qt = nc.dram_tensor("qt", [SL, 64, T], BF16, kind="ExternalInput")
    lkt = nc.dram_tensor("lkt", [SL, 64, LKT_W], BF16, kind="ExternalInput")
    skt = nc.dram_tensor("skt", [SL, 64, SKT_W], BF16, kind="ExternalInput")
    gkt = nc.dram_tensor("gkt", [SL, 64, 128], BF16, kind="ExternalInput")
    lv = nc.dram_tensor("lv", [SL, 128, LV_C * 65], BF16, kind="ExternalInput")
    sv = nc.dram_tensor("sv", [SL, 128, 4 * SV_C * 65], BF16, kind="ExternalInput")
    gv = nc.dram_tensor("gv", [SL, 128, 65], BF16, kind="ExternalInput")
    ident = nc.dram_tensor("ident", [65, 65], BF16, kind="ExternalInput")
    o = nc.dram_tensor("o", [SL, T, D], F32, kind="ExternalOutput")

    EXP = mybir.ActivationFunctionType.Exp

    with ExitStack() as es:
        ec = es.enter_context
        # double-buffered inputs (slot parity)
        qt_t = [ec(nc.sbuf_tensor(f"qt_t{i}", [64, T], BF16)) for i in range(2)]
        lkt_t = [ec(nc.sbuf_tensor(f"lkt_t{i}", [64, LKT_W], BF16)) for i in range(2)]
        skt_t = [ec(nc.sbuf_tensor(f"skt_t{i}", [64, SKT_W], BF16)) for i in range(2)]
        gkt_t = [ec(nc.sbuf_tensor(f"gkt_t{i}", [64, 128], BF16)) for i in range(2)]
        lv_t = [ec(nc.sbuf_tensor(f"lv_t{i}", [128, LV_C * 65], BF16)) for i in range(2)]
        sv_t = [ec(nc.sbuf_tensor(f"sv_t{i}", [128, 4 * SV_C * 65], BF16)) for i in range(2)]
        gv_t = [ec(nc.sbuf_tensor(f"gv_t{i}", [128, 65], BF16)) for i in range(2)]
        # double-buffered per-pair working set (pair parity)
        psS = [ec(nc.psum_tensor(f"psS{i}", [128, 1536], F32)) for i in range(2)]  # 3 banks
        # pvx bank layout: cols 0:256 = pvT accumulator [65p, 2 blocks x q],
        # cols 256:321 / 384:449 = transposed per-block [q, V|Z]
        pvx = [ec(nc.psum_tensor(f"pvx{i}", [128, 512], F32)) for i in range(2)]   # 1 bank
        pp = [ec(nc.sbuf_tensor(f"pp{i}", [128, 1536], BF16)) for i in range(2)]
        pvT_sb = [ec(nc.sbuf_tensor(f"pvT_sb{i}", [65, 256], BF16)) for i in range(2)]
        ident_t = ec(nc.sbuf_tensor("ident_t", [65, 65], BF16))
        rec = [ec(nc.sbuf_tensor(f"rec{i}", [128, 2], F32)) for i in range(2)]
        ob = [ec(nc.sbuf_tensor(f"ob{i}", [128, 128], F32)) for i in range(2)]

        di = [ec(nc.semaphore(f"di{i}")) for i in range(2)]  # input loads, slot parity
        st = [ec(nc.semaphore(f"st{i}")) for i in range(2)]  # out stores, pair parity
        pe_s = ec(nc.semaphore("pe_s"))  # +1 per pair: score matmuls done
        pe_v = ec(nc.semaphore("pe_v"))  # +1 per pair: PV matmuls done
        act = ec(nc.semaphore("act"))    # +1 per pair: exp done
        cpy = ec(nc.semaphore("cpy"))    # +1 per pair: pvT copied to SBUF
        pe_t = ec(nc.semaphore("pe_t"))  # +1 per pair: transpose done
        dve = ec(nc.semaphore("dve"))    # +1 per pair: normalize done
        block = ec(nc.Block())

        @block.sync
        def _(sync):
            def load_slot(s):
                u = s % 2
                for dst, src in (
                    (qt_t[u], qt[s]),
                    (lkt_t[u], lkt[s]),
                    (skt_t[u], skt[s]),
                    (gkt_t[u], gkt[s]),
                    (lv_t[u], lv[s]),
                    (sv_t[u], sv[s]),
                    (gv_t[u], gv[s]),
                ):
                    sync.dma_start(dst[:], src).then_inc(di[u], 16)

            sync.dma_start(ident_t[:], ident[:]).then_inc(di[0], 16)
            load_slot(0)
            load_slot(1)
            for p in range(NP):
                s, hb = divmod(p, PPS)
                b = 2 * hb
                u = p % 2
                sync.dma_start(
                    o[s, b * B : (b + 1) * B, :], ob[u][:, 0:64]
                ).wait_op(dve, p + 1, GE).then_inc(st[u], 16)
                sync.dma_start(
                    o[s, (b + 1) * B : (b + 2) * B, :], ob[u][:, 64:128]
                ).then_inc(st[u], 16)
                if hb == PPS - 1 and s + 2 < SL:
                    load_slot(s + 2)
            sync.wait_ge(st[0], 16 * NP)
            sync.wait_ge(st[1], 16 * NP)

        def emit_scores(p):
            s, hb = divmod(p, PPS)
            b = 2 * hb
            u = p % 2
            su = s % 2
            if hb == 0:
                nc.tensor.wait_ge(
                    di[su], 7 * 16 * (s // 2 + 1) + (16 if su == 0 else 0)
                )
            qA = qt_t[su][:, b * B : (b + 1) * B]
            qB = qt_t[su][:, (b + 1) * B : (b + 2) * B]
            qAB = qt_t[su][:, b * B : (b + 2) * B]
            w1a, w2a = 32 * b, 32 * b + 224
            w1b, w2b = w1a + 32, w2a + 32
            mms = (
                (C_SP1A, 128, skt_t[su][:, w1a : w1a + 128], qA),
                (C_SP1B, 128, skt_t[su][:, w1b : w1b + 128], qB),
                (C_SP2A, 128, skt_t[su][:, w2a : w2a + 128], qA),
                (C_SP2B, 128, skt_t[su][:, w2b : w2b + 128], qB),
                (C_G, 256, gkt_t[su][:, :], qAB),
                (C_LOC1, 256, lkt_t[su][:, (b + 1) * B : (b + 2) * B], qAB),
                (C_LOC0, 128, lkt_t[su][:, b * B : (b + 1) * B], qA),
                (C_LOC2, 256, lkt_t[su][:, (b + 2) * B : (b + 3) * B], qAB),
                (C_LOC3, 128, lkt_t[su][:, (b + 3) * B : (b + 4) * B], qB),
            )
            for kk, (col, w, lhsT, rhs) in enumerate(mms):
                mm = nc.tensor.matmul(
                    psS[u][:, col : col + w], lhsT, rhs, start=True, stop=True
                )
                if kk == len(mms) - 1:
                    mm.then_inc(pe_s, 1)

        def emit_pv(p):
            s, hb = divmod(p, PPS)
            b = 2 * hb
            u = p % 2
            su = s % 2
            if p >= 2:
                nc.tensor.wait_ge(dve, p - 1)  # pvx[u] free
            bA, bB = b, b + 1

            def svs(bb, w):
                start = 32 * bb + (0 if w == 1 else 224)
                c, r = divmod(start, 128)
                off = ((r // 32) * SV_C + c) * 65
                return sv_t[su][:, off : off + 65]

            # out cols: block A q = 0:128, block B q = 128:256; g first with
            # start=True covers the whole range, the rest accumulate
            mms = (
                (gv_t[su][:], C_G, 256, 0),
                (svs(bA, 1), C_SP1A, 128, 0),
                (svs(bB, 1), C_SP1B, 128, 128),
                (svs(bA, 2), C_SP2A, 128, 0),
                (svs(bB, 2), C_SP2B, 128, 128),
                (lv_t[su][:, (b + 1) * 65 : (b + 2) * 65], C_LOC1, 256, 0),
                (lv_t[su][:, b * 65 : (b + 1) * 65], C_LOC0, 128, 0),
                (lv_t[su][:, (b + 2) * 65 : (b + 3) * 65], C_LOC2, 256, 0),
                (lv_t[su][:, (b + 3) * 65 : (b + 4) * 65], C_LOC3, 128, 128),
            )
            for kk, (vch, ppcol, w, ocol) in enumerate(mms):
                mm = nc.tensor.matmul(
                    pvx[u][0:65, ocol : ocol + w],
                    vch,
                    pp[u][:, ppcol : ppcol + w],
                    start=(kk == 0), stop=(kk == len(mms) - 1),
                    skip_group_check=True,
                )
                if kk == 0:
                    mm.wait_op(act, p + 1, GE)  # pp[u] ready
                if kk == len(mms) - 1:
                    mm.then_inc(pe_v, 1)

        def emit_transpose(p):
            u = p % 2
            mm = nc.tensor.matmul(
                pvx[u][:, 256:321], pvT_sb[u][:, 0:128], ident_t[:],
                start=True, stop=True,
            )
            mm.wait_op(cpy, p + 1, GE)  # pvT_sb[u] ready
            nc.tensor.matmul(
                pvx[u][:, 384:449], pvT_sb[u][:, 128:256], ident_t[:],
                start=True, stop=True,
            ).then_inc(pe_t, 1)

        @block.tensor
        def _(tensor):
            emit_scores(0)
            emit_scores(1)
            for p in range(NP):
                emit_pv(p)
                if p + 2 < NP:
                    emit_scores(p + 2)
                emit_transpose(p)

        @block.scalar
        def _(scalar):
            for p in range(NP):
                u = p % 2
                if p >= 2:
                    scalar.wait_ge(pe_v, p - 1)  # pp[u] free: PV of p-2 done
                nc.scalar.activation(
                    pp[u][:], psS[u][:, 0:1536], EXP, scale=0.125
                ).wait_op(pe_s, p + 1, GE).then_inc(act, 1)

        @block.vector
        def _(vector):
            for p in range(NP):
                u = p % 2
                if p >= 2:
                    vector.wait_ge(st[u], 32 * (p // 2))  # ob[u]/rec[u] free
                nc.vector.tensor_copy(
                    pvT_sb[u][:], pvx[u][0:65, 0:256]
                ).wait_op(pe_v, p + 1, GE).then_inc(cpy, 1)
                nc.vector.reciprocal(rec[u][:, 0:1], pvx[u][:, 320:321]).wait_op(
                    pe_t, p + 1, GE
                )
                nc.vector.reciprocal(rec[u][:, 1:2], pvx[u][:, 448:449])
                nc.vector.drain()  # DVE pipeline RAW: rec written, read next
                nc.vector.tensor_mul(
                    ob[u][:, 0:64], pvx[u][:, 256:320],
                    rec[u][:, 0:1].broadcast_to([128, 64]),
                )
                nc.vector.tensor_mul(
                    ob[u][:, 64:128], pvx[u][:, 384:448],
                    rec[u][:, 1:2].broadcast_to([128, 64]),
                ).then_inc(dve, 1)

    return nc


def _prepare(inputs):
    import ml_dtypes

    bf = ml_dtypes.bfloat16
    f = np.float32
    q = np.asarray(inputs["query_layer"], f).reshape(NH, T, D)
    k = np.asarray(inputs["key_layer"], f).reshape(NH, T, D)
    v = np.asarray(inputs["value_layer"], f).reshape(NH, T, D)
    sk = np.asarray(inputs["sparse_key"], f).reshape(NH, TSP, D)
    svv = np.asarray(inputs["sparse_value"], f).reshape(NH, TSP, D)
    gk = np.asarray(inputs["global_key"], f).reshape(NH, G, D)
    gvv = np.asarray(inputs["global_value"], f).reshape(NH, G, D)
    am = np.repeat(np.asarray(inputs["attention_mask"], f)[:, 0, 0, :], H, 0)
    sm = np.repeat(np.asarray(inputs["sparse_mask"], f)[:, 0, 0, :], H, 0)
    gm = np.repeat(np.asarray(inputs["global_mask"], f)[:, 0, 0, :], H, 0)

    qt = np.ascontiguousarray(q.transpose(0, 2, 1)).astype(bf)

    lkt = np.zeros((NH, 64, LKT_W), f)
    lkt[:, :, B : B + T] = k.transpose(0, 2, 1)
    lkt = lkt.astype(bf)

    skt = np.zeros((NH, 64, SKT_W), f)
    skt[:, :, 160 : 160 + TSP] = sk.transpose(0, 2, 1)
    skt = skt.astype(bf)

    gkt = np.zeros((NH, 64, 128), f)
    gkt[:, :, :G] = gk.transpose(0, 2, 1)
    gkt = gkt.astype(bf)

    # V_aug rows scaled by exp(mask); pad rows are all-zero
    em_l = np.zeros((NH, LKT_W), f)
    em_l[:, B : B + T] = np.exp(am)
    lvp = np.zeros((NH, LKT_W, 65), f)
    lvp[:, B : B + T, :64] = v
    lvp[:, :, 64] = 1.0
    lvp *= em_l[:, :, None]
    lvp = np.ascontiguousarray(
        lvp.reshape(NH, LV_C, 128, 65).transpose(0, 2, 1, 3)
    ).reshape(NH, 128, LV_C * 65).astype(bf)

    SVP_W = 96 + SV_C * 128
    em_s = np.zeros((NH, SVP_W), f)
    em_s[:, 160 : 160 + TSP] = np.exp(sm)
    sv_pad = np.zeros((NH, SVP_W, 65), f)
    sv_pad[:, 160 : 160 + TSP, :64] = svv
    sv_pad[:, :, 64] = 1.0
    sv_pad *= em_s[:, :, None]
    svp = np.empty((NH, 4, 128, SV_C, 65), f)
    for p in range(4):
        svp[:, p] = (
            sv_pad[:, 32 * p : 32 * p + SV_C * 128]
            .reshape(NH, SV_C, 128, 65)
            .transpose(0, 2, 1, 3)
        )
    svp = np.ascontiguousarray(svp.transpose(0, 2, 1, 3, 4)).reshape(
        NH, 128, 4 * SV_C * 65
    ).astype(bf)

    gvp = np.zeros((NH, 128, 65), f)
    gvp[:, :G, :64] = gvv
    gvp[:, :G, 64] = 1.0
    gvp[:, :G] *= np.exp(gm)[:, :, None]
    gvp = gvp.astype(bf)
    identm = np.eye(65, dtype=bf)

    return [
        {
            "ident": identm,
            "qt": qt[c * SL : (c + 1) * SL],
            "lkt": lkt[c * SL : (c + 1) * SL],
            "skt": skt[c * SL : (c + 1) * SL],
            "gkt": gkt[c * SL : (c + 1) * SL],
            "lv": lvp[c * SL : (c + 1) * SL],
            "sv": svp[c * SL : (c + 1) * SL],
            "gv": gvp[c * SL : (c + 1) * SL],
        }
        for c in range(NCORES)
    ]


_NC_CACHE = {}
LAST_RESULTS = None


def kernel(**inputs):
    global LAST_RESULTS
    if "nc" not in _NC_CACHE:
        _NC_CACHE["nc"] = _build_bass()
    nc = _NC_CACHE["nc"]
    in_maps = _prepare(inputs)
    res = run_bass_kernel_spmd(nc, in_maps, core_ids=list(range(NCORES)))
    LAST_RESULTS = res
    out = np.empty((NH, T, D), np.float32)
    for c in range(NCORES):
        out[c * SL : (c + 1) * SL] = res.results[c]["o"]
    return out.reshape(N, H, T, D)


# revision 13
# speedup vs baseline: 1.2081x; 1.2081x over previous
"""Block-local sparse attention (LSG-style) on 8 TRN2 NeuronCores.

Sharding: the 32 (n, h) pairs are split 4-per-core (data/head parallel, no
collectives). Host-side numpy prep re-lays-out the inputs so the device
kernel needs no transposes, all bf16:

  - qt : Q^T per head [64, T]
  - lkt/skt/gkt: local/sparse/global K^T, token-padded with zeros
  - lv/sv/gv: V with a ones column appended (col 64), chunked [128, c, 65],
    and every row scaled by exp(mask): softmax(QK/8 + m) @ V is computed as
    sum_t exp(s_t) e^{m_t} [V_t, 1], then a divide by the accumulated last
    column — exact for any additive mask, and pad tokens (e^{m}=0) vanish
    from both numerator and denominator, so no mask row and no
    max-subtraction are needed (|QK|/8 is O(5), well within fp32 exp range).
    sv additionally holds 4 phase-shifted copies so the 32-token-granular
    sparse windows always start at partition 0.

The device processes query-block PAIRS: 9 score matmuls per pair (the
shared global chunk and the two shared local chunks stream both blocks'
256 q columns in one matmul) into a 3-bank PSUM region [128, 1536] laid
out so no matmul output crosses a bank, one wide exp(S/8) on ACT, then
12 PV matmuls (6 per block, N=65) accumulating [q, V|Z] and a
reciprocal-normalize on DVE.

Raw bass with hand-placed semaphores: this walrus build encodes at most one
sem wait per matmul/ACT instruction, which rules out TileContext (its
scheduler attaches several). The pipeline is software-pipelined by hand
(scores of pair p+2 issue right after PV of pair p so exp overlaps PE) with
double-buffered SBUF/PSUM and parity-split counting semaphores (a DMA batch
and its grandparent share a buffer; per-parity sems make "mine completed"
exact even if DMA queues complete out of order).
"""

from contextlib import ExitStack

import numpy as np

import concourse.bass as bass
import concourse.mybir as mybir
from concourse.bass_utils import run_bass_kernel_spmd

N, H, T, D = 2, 16, 4096, 64
B = 128          # query block
NB = T // B      # 32
G = 64           # global tokens
TSP = T // 4     # sparse tokens (1024)
NH = N * H       # 32
NCORES = 8
SL = NH // NCORES  # 4 heads per core
NP = SL * NB // 2  # 64 block-pairs per core
PPS = NB // 2      # 16 pairs per slot

LKT_W = T + 2 * B            # 4352 padded local tokens
SKT_W = TSP + 320            # 1344 padded sparse tokens
LV_C = LKT_W // 128          # 34 local V chunks
SV_C = 11                    # sparse V chunks per phase

F32 = mybir.dt.float32
BF16 = mybir.dt.bfloat16
GE = "sem-ge"

# column layout of the per-pair score/prob tile [128, 1536] (3 PSUM banks;
# regions never cross a 512-col bank boundary)
C_SP1A, C_SP1B = 0, 128
C_SP2A, C_SP2B = 256, 384
C_G = 512        # 256 wide: q of both blocks
C_LOC1 = 768     # 256 wide: local chunk b+1, both blocks
C_LOC0 = 1024    # 128: local chunk b, block A only
C_LOC2 = 1152    # 256 wide: local chunk b+2, both blocks
C_LOC3 = 1408    # 128: local chunk b+3, block B only


def _build_bass():
    nc = bass.Bass("TRN2", num_devices=NCORES, debug=False)

    qt = nc.dram_tensor("qt", [SL, 128, T], BF16, kind="ExternalInput")
    lkt = nc.dram_tensor("lkt", [SL, 128, LKT_W], BF16, kind="ExternalInput")
    skt = nc.dram_tensor("skt", [SL, 128, SKT_W], BF16, kind="ExternalInput")
    gkt = nc.dram_tensor("gkt", [SL, 128, 128], BF16, kind="ExternalInput")
    lv = nc.dram_tensor("lv", [SL, 128, LV_C * 65], BF16, kind="ExternalInput")
    sv = nc.dram_tensor("sv", [SL, 128, 4 * SV_C * 65], BF16, kind="ExternalInput")
    gv = nc.dram_tensor("gv", [SL, 128, 65], BF16, kind="ExternalInput")
    o = nc.dram_tensor("o", [SL, T, D], F32, kind="ExternalOutput")

    EXP = mybir.ActivationFunctionType.Exp

    with ExitStack() as es:
        ec = es.enter_context
        # double-buffered inputs (slot parity)
        qt_t = [ec(nc.sbuf_tensor(f"qt_t{i}", [128, T], BF16)) for i in range(2)]
        lkt_t = [ec(nc.sbuf_tensor(f"lkt_t{i}", [128, LKT_W], BF16)) for i in range(2)]
        skt_t = [ec(nc.sbuf_tensor(f"skt_t{i}", [128, SKT_W], BF16)) for i in range(2)]
        gkt_t = [ec(nc.sbuf_tensor(f"gkt_t{i}", [128, 128], BF16)) for i in range(2)]
        lv_t = [ec(nc.sbuf_tensor(f"lv_t{i}", [128, LV_C * 65], BF16)) for i in range(2)]
        sv_t = [ec(nc.sbuf_tensor(f"sv_t{i}", [128, 4 * SV_C * 65], BF16)) for i in range(2)]
        gv_t = [ec(nc.sbuf_tensor(f"gv_t{i}", [128, 65], BF16)) for i in range(2)]
        # double-buffered per-pair working set (pair parity)
        psS = [ec(nc.psum_tensor(f"psS{i}", [128, 1536], F32)) for i in range(2)]  # 3 banks
        pv = [ec(nc.psum_tensor(f"pv{i}", [128, 512], F32)) for i in range(2)]     # 1 bank
        pp = [ec(nc.sbuf_tensor(f"pp{i}", [128, 1536], BF16)) for i in range(2)]
        rec = [ec(nc.sbuf_tensor(f"rec{i}", [128, 2], F32)) for i in range(2)]
        ob = [ec(nc.sbuf_tensor(f"ob{i}", [128, 128], F32)) for i in range(2)]

        di = [ec(nc.semaphore(f"di{i}")) for i in range(2)]  # input loads, slot parity
        st = [ec(nc.semaphore(f"st{i}")) for i in range(2)]  # out stores, pair parity
        pe_s = ec(nc.semaphore("pe_s"))  # +1 per pair: score matmuls done
        pe_v = ec(nc.semaphore("pe_v"))  # +1 per pair: PV matmuls done
        act = ec(nc.semaphore("act"))    # +1 per pair: exp done
        dve = ec(nc.semaphore("dve"))    # +1 per pair: normalize done
        block = ec(nc.Block())

        @block.sync
        def _(sync):
            def load_slot(s):
                u = s % 2
                for dst, src in (
                    (qt_t[u], qt[s]),
                    (lkt_t[u], lkt[s]),
                    (skt_t[u], skt[s]),
                    (gkt_t[u], gkt[s]),
                    (lv_t[u], lv[s]),
                    (sv_t[u], sv[s]),
                    (gv_t[u], gv[s]),
                ):
                    sync.dma_start(dst[:], src).then_inc(di[u], 16)

            load_slot(0)
            load_slot(1)
            for p in range(NP):
                s, hb = divmod(p, PPS)
                b = 2 * hb
                u = p % 2
                sync.dma_start(
                    o[s, b * B : (b + 1) * B, :], ob[u][:, 0:64]
                ).wait_op(dve, p + 1, GE).then_inc(st[u], 16)
                sync.dma_start(
                    o[s, (b + 1) * B : (b + 2) * B, :], ob[u][:, 64:128]
                ).then_inc(st[u], 16)
                if hb == PPS - 1 and s + 2 < SL:
                    load_slot(s + 2)
            sync.wait_ge(st[0], 16 * NP)
            sync.wait_ge(st[1], 16 * NP)

        def emit_scores(p):
            s, hb = divmod(p, PPS)
            b = 2 * hb
            u = p % 2
            su = s % 2
            if hb == 0:
                nc.tensor.wait_ge(di[su], 7 * 16 * (s // 2 + 1))
            qA = qt_t[su][:, b * B : (b + 1) * B]
            qB = qt_t[su][:, (b + 1) * B : (b + 2) * B]
            qAB = qt_t[su][:, b * B : (b + 2) * B]
            w1a, w2a = 32 * b, 32 * b + 224
            w1b, w2b = w1a + 32, w2a + 32
            mms = (
                (C_SP1A, 128, skt_t[su][:, w1a : w1a + 128], qA),
                (C_SP1B, 128, skt_t[su][:, w1b : w1b + 128], qB),
                (C_SP2A, 128, skt_t[su][:, w2a : w2a + 128], qA),
                (C_SP2B, 128, skt_t[su][:, w2b : w2b + 128], qB),
                (C_G, 256, gkt_t[su][:, :], qAB),
                (C_LOC1, 256, lkt_t[su][:, (b + 1) * B : (b + 2) * B], qAB),
                (C_LOC0, 128, lkt_t[su][:, b * B : (b + 1) * B], qA),
                (C_LOC2, 256, lkt_t[su][:, (b + 2) * B : (b + 3) * B], qAB),
                (C_LOC3, 128, lkt_t[su][:, (b + 3) * B : (b + 4) * B], qB),
            )
            for kk, (col, w, lhsT, rhs) in enumerate(mms):
                mm = nc.tensor.matmul(
                    psS[u][:, col : col + w],
                    lhsT[0:64, :], rhs[0:64, :],
                    start=True, stop=True,
                )
                if kk == len(mms) - 1:
                    mm.then_inc(pe_s, 1)

        def emit_pv(p):
            s, hb = divmod(p, PPS)
            b = 2 * hb
            u = p % 2
            su = s % 2
            if p >= 2:
                nc.tensor.wait_ge(dve, p - 1)  # pv[u] free
            first = True
            for blk in range(2):
                bb = b + blk
                w1, w2 = 32 * bb, 32 * bb + 224
                c1, r1 = divmod(w1, 128)
                c2, r2 = divmod(w2, 128)
                sp1c = ((r1 // 32) * SV_C + c1) * 65
                sp2c = ((r2 // 32) * SV_C + c2) * 65
                if blk == 0:
                    lhs = (C_SP1A, C_SP2A, C_G, C_LOC0, C_LOC1, C_LOC2)
                else:
                    lhs = (C_SP1B, C_SP2B, C_G + 128, C_LOC1 + 128,
                           C_LOC2 + 128, C_LOC3)
                rhss = (
                    sv_t[su][:, sp1c : sp1c + 65],
                    sv_t[su][:, sp2c : sp2c + 65],
                    gv_t[su][:],
                    lv_t[su][:, bb * 65 : bb * 65 + 65],
                    lv_t[su][:, (bb + 1) * 65 : (bb + 1) * 65 + 65],
                    lv_t[su][:, (bb + 2) * 65 : (bb + 2) * 65 + 65],
                )
                out = pv[u][:, blk * 128 : blk * 128 + 65]
                for j in range(6):
                    mm = nc.tensor.matmul(
                        out, pp[u][:, lhs[j] : lhs[j] + 128], rhss[j],
                        start=(j == 0), stop=(j == 5),
                    )
                    if first:
                        mm.wait_op(act, p + 1, GE)  # pp[u] ready
                        first = False
                    if blk == 1 and j == 5:
                        mm.then_inc(pe_v, 1)

        @block.tensor
        def _(tensor):
            emit_scores(0)
            emit_scores(1)
            for p in range(NP):
                emit_pv(p)
                if p + 2 < NP:
                    emit_scores(p + 2)

        @block.scalar
        def _(scalar):
            for p in range(NP):
                u = p % 2
                if p >= 2:
                    scalar.wait_ge(pe_v, p - 1)  # pp[u] free: PV of p-2 done
                nc.scalar.activation(
                    pp[u][:], psS[u][:, 0:1536], EXP, scale=0.125
                ).wait_op(pe_s, p + 1, GE).then_inc(act, 1)

        @block.vector
        def _(vector):
            for p in range(NP):
                u = p % 2
                if p >= 2:
                    vector.wait_ge(st[u], 32 * (p // 2))  # ob[u]/rec[u] free
                nc.vector.reciprocal(rec[u][:, 0:1], pv[u][:, 64:65]).wait_op(
                    pe_v, p + 1, GE
                )
                nc.vector.reciprocal(rec[u][:, 1:2], pv[u][:, 192:193])
                nc.vector.drain()  # DVE pipeline RAW: rec written, read next
                nc.vector.tensor_mul(
                    ob[u][:, 0:64], pv[u][:, 0:64],
                    rec[u][:, 0:1].broadcast_to([128, 64]),
                )
                nc.vector.tensor_mul(
                    ob[u][:, 64:128], pv[u][:, 128:192],
                    rec[u][:, 1:2].broadcast_to([128, 64]),
                ).then_inc(dve, 1)

    return nc


def _prepare(inputs):
    import ml_dtypes

    bf = ml_dtypes.bfloat16
    f = np.float32
    q = np.asarray(inputs["query_layer"], f).reshape(NH, T, D)
    k = np.asarray(inputs["key_layer"], f).reshape(NH, T, D)
    v = np.asarray(inputs["value_layer"], f).reshape(NH, T, D)
    sk = np.asarray(inputs["sparse_key"], f).reshape(NH, TSP, D)
    svv = np.asarray(inputs["sparse_value"], f).reshape(NH, TSP, D)
    gk = np.asarray(inputs["global_key"], f).reshape(NH, G, D)
    gvv = np.asarray(inputs["global_value"], f).reshape(NH, G, D)
    am = np.repeat(np.asarray(inputs["attention_mask"], f)[:, 0, 0, :], H, 0)
    sm = np.repeat(np.asarray(inputs["sparse_mask"], f)[:, 0, 0, :], H, 0)
    gm = np.repeat(np.asarray(inputs["global_mask"], f)[:, 0, 0, :], H, 0)

    qt = np.ascontiguousarray(
        np.concatenate([q.transpose(0, 2, 1)] * 2, axis=1)
    ).astype(bf)

    lkt = np.zeros((NH, 128, LKT_W), f)
    lkt[:, :64, B : B + T] = k.transpose(0, 2, 1)
    lkt[:, 64:] = lkt[:, :64]
    lkt = lkt.astype(bf)

    skt = np.zeros((NH, 128, SKT_W), f)
    skt[:, :64, 160 : 160 + TSP] = sk.transpose(0, 2, 1)
    skt[:, 64:] = skt[:, :64]
    skt = skt.astype(bf)

    gkt = np.zeros((NH, 128, 128), f)
    gkt[:, :64, :G] = gk.transpose(0, 2, 1)
    gkt[:, 64:] = gkt[:, :64]
    gkt = gkt.astype(bf)

    # V_aug rows scaled by exp(mask); pad rows are all-zero
    em_l = np.zeros((NH, LKT_W), f)
    em_l[:, B : B + T] = np.exp(am)
    lvp = np.zeros((NH, LKT_W, 65), f)
    lvp[:, B : B + T, :64] = v
    lvp[:, :, 64] = 1.0
    lvp *= em_l[:, :, None]
    lvp = np.ascontiguousarray(
        lvp.reshape(NH, LV_C, 128, 65).transpose(0, 2, 1, 3)
    ).reshape(NH, 128, LV_C * 65).astype(bf)

    SVP_W = 96 + SV_C * 128
    em_s = np.zeros((NH, SVP_W), f)
    em_s[:, 160 : 160 + TSP] = np.exp(sm)
    sv_pad = np.zeros((NH, SVP_W, 65), f)
    sv_pad[:, 160 : 160 + TSP, :64] = svv
    sv_pad[:, :, 64] = 1.0
    sv_pad *= em_s[:, :, None]
    svp = np.empty((NH, 4, 128, SV_C, 65), f)
    for p in range(4):
        svp[:, p] = (
            sv_pad[:, 32 * p : 32 * p + SV_C * 128]
            .reshape(NH, SV_C, 128, 65)
            .transpose(0, 2, 1, 3)
        )
    svp = np.ascontiguousarray(svp.transpose(0, 2, 1, 3, 4)).reshape(
        NH, 128, 4 * SV_C * 65
    ).astype(bf)

    gvp = np.zeros((NH, 128, 65), f)
    gvp[:, :G, :64] = gvv
    gvp[:, :G, 64] = 1.0
    gvp[:, :G] *= np.exp(gm)[:, :, None]
    gvp = gvp.astype(bf)

    return [
        {
            "qt": qt[c * SL : (c + 1) * SL],
            "lkt": lkt[c * SL : (c + 1) * SL],
            "skt": skt[c * SL : (c + 1) * SL],
            "gkt": gkt[c * SL : (c + 1) * SL],
            "lv": lvp[c * SL : (c + 1) * SL],
            "sv": svp[c * SL : (c + 1) * SL],
            "gv": gvp[c * SL : (c + 1) * SL],
        }
        for c in range(NCORES)
    ]


_NC_CACHE = {}
LAST_RESULTS = None


def kernel(**inputs):
    global LAST_RESULTS
    if "nc" not in _NC_CACHE:
        _NC_CACHE["nc"] = _build_bass()
    nc = _NC_CACHE["nc"]
    in_maps = _prepare(inputs)
    res = run_bass_kernel_spmd(nc, in_maps, core_ids=list(range(NCORES)))
    LAST_RESULTS = res
    out = np.empty((NH, T, D), np.float32)
    for c in range(NCORES):
        out[c * SL : (c + 1) * SL] = res.results[c]["o"]
    return out.reshape(N, H, T, D)


# revision 14
# speedup vs baseline: 1.2770x; 1.0570x over previous
"""Block-local sparse attention (LSG-style) on 8 TRN2 NeuronCores.

Sharding: the 32 (n, h) pairs are split 4-per-core (data/head parallel, no
collectives). Host-side numpy prep re-lays-out the inputs so the device
kernel needs no transposes, all bf16:

  - qt : Q^T per head [64, T]
  - lkt/skt/gkt: local/sparse/global K^T, token-padded with zeros
  - lv/sv/gv: V with a ones column appended (col 64), chunked [128, c, 65],
    and every row scaled by exp(mask): softmax(QK/8 + m) @ V is computed as
    sum_t exp(s_t) e^{m_t} [V_t, 1], then a divide by the accumulated last
    column — exact for any additive mask, and pad tokens (e^{m}=0) vanish
    from both numerator and denominator, so no mask row and no
    max-subtraction are needed (|QK|/8 is O(5), well within fp32 exp range).
    sv additionally holds 4 phase-shifted copies so the 32-token-granular
    sparse windows always start at partition 0.

The device processes query-block PAIRS: 9 score matmuls per pair (the
shared global chunk and the two shared local chunks stream both blocks'
256 q columns in one matmul) into a 3-bank PSUM region [128, 1536] laid
out so no matmul output crosses a bank, one wide exp(S/8) on ACT, then
12 PV matmuls (6 per block, N=65) accumulating [q, V|Z] and a
reciprocal-normalize on DVE.

Raw bass with hand-placed semaphores: this walrus build encodes at most one
sem wait per matmul/ACT instruction, which rules out TileContext (its
scheduler attaches several). The pipeline is software-pipelined by hand
(scores of pair p+2 issue right after PV of pair p so exp overlaps PE) with
double-buffered SBUF/PSUM and parity-split counting semaphores (a DMA batch
and its grandparent share a buffer; per-parity sems make "mine completed"
exact even if DMA queues complete out of order).
"""

from contextlib import ExitStack

import numpy as np

import concourse.bass as bass
import concourse.mybir as mybir
from concourse.bass_utils import run_bass_kernel_spmd

N, H, T, D = 2, 16, 4096, 64
B = 128          # query block
NB = T // B      # 32
G = 64           # global tokens
TSP = T // 4     # sparse tokens (1024)
NH = N * H       # 32
NCORES = 8
SL = NH // NCORES  # 4 heads per core
NP = SL * NB // 2  # 64 block-pairs per core
PPS = NB // 2      # 16 pairs per slot

LKT_W = T + 2 * B            # 4352 padded local tokens
SKT_W = TSP + 320            # 1344 padded sparse tokens
LV_C = LKT_W // 128          # 34 local V chunks
SV_C = 11                    # sparse V chunks per phase

F32 = mybir.dt.float32
BF16 = mybir.dt.bfloat16
GE = "sem-ge"

# column layout of the per-pair score/prob tile [128, 1536] (3 PSUM banks;
# regions never cross a 512-col bank boundary)
C_SP1A, C_SP1B = 0, 128
C_SP2A, C_SP2B = 256, 384
C_G = 512        # 256 wide: q of both blocks
C_LOC1 = 768     # 256 wide: local chunk b+1, both blocks
C_LOC0 = 1024    # 128: local chunk b, block A only
C_LOC2 = 1152    # 256 wide: local chunk b+2, both blocks
C_LOC3 = 1408    # 128: local chunk b+3, block B only


def _build_bass():
    nc = bass.Bass("TRN2", num_devices=NCORES, debug=False)

    qt = nc.dram_tensor("qt", [SL, 64, T], BF16, kind="ExternalInput")
    lkt = nc.dram_tensor("lkt", [SL, 64, LKT_W], BF16, kind="ExternalInput")
    skt = nc.dram_tensor("skt", [SL, 64, SKT_W], BF16, kind="ExternalInput")
    gkt = nc.dram_tensor("gkt", [SL, 64, 128], BF16, kind="ExternalInput")
    lv = nc.dram_tensor("lv", [SL, 128, LV_C * 65], BF16, kind="ExternalInput")
    sv = nc.dram_tensor("sv", [SL, 128, 4 * SV_C * 65], BF16, kind="ExternalInput")
    gv = nc.dram_tensor("gv", [SL, 128, 65], BF16, kind="ExternalInput")
    o = nc.dram_tensor("o", [SL, T, D], F32, kind="ExternalOutput")

    EXP = mybir.ActivationFunctionType.Exp

    with ExitStack() as es:
        ec = es.enter_context
        # double-buffered inputs (slot parity)
        qt_t = [ec(nc.sbuf_tensor(f"qt_t{i}", [64, T], BF16)) for i in range(2)]
        lkt_t = [ec(nc.sbuf_tensor(f"lkt_t{i}", [64, LKT_W], BF16)) for i in range(2)]
        skt_t = [ec(nc.sbuf_tensor(f"skt_t{i}", [64, SKT_W], BF16)) for i in range(2)]
        gkt_t = [ec(nc.sbuf_tensor(f"gkt_t{i}", [64, 128], BF16)) for i in range(2)]
        lv_t = [ec(nc.sbuf_tensor(f"lv_t{i}", [128, LV_C * 65], BF16)) for i in range(2)]
        sv_t = [ec(nc.sbuf_tensor(f"sv_t{i}", [128, 4 * SV_C * 65], BF16)) for i in range(2)]
        gv_t = [ec(nc.sbuf_tensor(f"gv_t{i}", [128, 65], BF16)) for i in range(2)]
        # double-buffered per-pair working set (pair parity)
        psS = [ec(nc.psum_tensor(f"psS{i}", [128, 1536], F32)) for i in range(2)]  # 3 banks
        pv = [ec(nc.psum_tensor(f"pv{i}", [128, 512], F32)) for i in range(2)]     # 1 bank
        pp = [ec(nc.sbuf_tensor(f"pp{i}", [128, 1536], BF16)) for i in range(2)]
        rec = [ec(nc.sbuf_tensor(f"rec{i}", [128, 2], F32)) for i in range(2)]
        ob = [ec(nc.sbuf_tensor(f"ob{i}", [128, 128], F32)) for i in range(2)]

        di = [ec(nc.semaphore(f"di{i}")) for i in range(2)]  # input loads, slot parity
        st = [ec(nc.semaphore(f"st{i}")) for i in range(2)]  # out stores, pair parity
        pe_s = ec(nc.semaphore("pe_s"))  # +1 per pair: score matmuls done
        pe_v = ec(nc.semaphore("pe_v"))  # +1 per pair: PV matmuls done
        act = ec(nc.semaphore("act"))    # +1 per pair: exp done
        dve = ec(nc.semaphore("dve"))    # +1 per pair: normalize done
        block = ec(nc.Block())

        @block.sync
        def _(sync):
            def load_slot(s):
                u = s % 2
                for dst, src in (
                    (qt_t[u], qt[s]),
                    (lkt_t[u], lkt[s]),
                    (skt_t[u], skt[s]),
                    (gkt_t[u], gkt[s]),
                    (lv_t[u], lv[s]),
                    (sv_t[u], sv[s]),
                    (gv_t[u], gv[s]),
                ):
                    sync.dma_start(dst[:], src).then_inc(di[u], 16)

            load_slot(0)
            load_slot(1)
            for p in range(NP):
                s, hb = divmod(p, PPS)
                b = 2 * hb
                u = p % 2
                sync.dma_start(
                    o[s, b * B : (b + 1) * B, :], ob[u][:, 0:64]
                ).wait_op(dve, p + 1, GE).then_inc(st[u], 16)
                sync.dma_start(
                    o[s, (b + 1) * B : (b + 2) * B, :], ob[u][:, 64:128]
                ).then_inc(st[u], 16)
                if hb == PPS - 1 and s + 2 < SL:
                    load_slot(s + 2)
            sync.wait_ge(st[0], 16 * NP)
            sync.wait_ge(st[1], 16 * NP)

        def emit_scores(p):
            s, hb = divmod(p, PPS)
            b = 2 * hb
            u = p % 2
            su = s % 2
            if hb == 0:
                nc.tensor.wait_ge(di[su], 7 * 16 * (s // 2 + 1))
            qA = qt_t[su][:, b * B : (b + 1) * B]
            qB = qt_t[su][:, (b + 1) * B : (b + 2) * B]
            qAB = qt_t[su][:, b * B : (b + 2) * B]
            w1a, w2a = 32 * b, 32 * b + 224
            w1b, w2b = w1a + 32, w2a + 32
            mms = (
                (C_SP1A, 128, skt_t[su][:, w1a : w1a + 128], qA),
                (C_SP1B, 128, skt_t[su][:, w1b : w1b + 128], qB),
                (C_SP2A, 128, skt_t[su][:, w2a : w2a + 128], qA),
                (C_SP2B, 128, skt_t[su][:, w2b : w2b + 128], qB),
                (C_G, 256, gkt_t[su][:, :], qAB),
                (C_LOC1, 256, lkt_t[su][:, (b + 1) * B : (b + 2) * B], qAB),
                (C_LOC0, 128, lkt_t[su][:, b * B : (b + 1) * B], qA),
                (C_LOC2, 256, lkt_t[su][:, (b + 2) * B : (b + 3) * B], qAB),
                (C_LOC3, 128, lkt_t[su][:, (b + 3) * B : (b + 4) * B], qB),
            )
            for kk, (col, w, lhsT, rhs) in enumerate(mms):
                mm = nc.tensor.matmul(
                    psS[u][:, col : col + w],
                    lhsT, rhs,
                    start=True, stop=True,
                )
                if kk == len(mms) - 1:
                    mm.then_inc(pe_s, 1)

        def emit_pv(p):
            s, hb = divmod(p, PPS)
            b = 2 * hb
            u = p % 2
            su = s % 2
            if p >= 2:
                nc.tensor.wait_ge(dve, p - 1)  # pv[u] free
            first = True
            for blk in range(2):
                bb = b + blk
                w1, w2 = 32 * bb, 32 * bb + 224
                c1, r1 = divmod(w1, 128)
                c2, r2 = divmod(w2, 128)
                sp1c = ((r1 // 32) * SV_C + c1) * 65
                sp2c = ((r2 // 32) * SV_C + c2) * 65
                if blk == 0:
                    lhs = (C_SP1A, C_SP2A, C_G, C_LOC0, C_LOC1, C_LOC2)
                else:
                    lhs = (C_SP1B, C_SP2B, C_G + 128, C_LOC1 + 128,
                           C_LOC2 + 128, C_LOC3)
                rhss = (
                    sv_t[su][:, sp1c : sp1c + 65],
                    sv_t[su][:, sp2c : sp2c + 65],
                    gv_t[su][:],
                    lv_t[su][:, bb * 65 : bb * 65 + 65],
                    lv_t[su][:, (bb + 1) * 65 : (bb + 1) * 65 + 65],
                    lv_t[su][:, (bb + 2) * 65 : (bb + 2) * 65 + 65],
                )
                out = pv[u][:, blk * 128 : blk * 128 + 65]
                for j in range(6):
                    mm = nc.tensor.matmul(
                        out, pp[u][:, lhs[j] : lhs[j] + 128], rhss[j],
                        start=(j == 0), stop=(j == 5),
                    )
                    if first:
                        mm.wait_op(act, p + 1, GE)  # pp[u] ready
                        first = False
                    if blk == 1 and j == 5:
                        mm.then_inc(pe_v, 1)

        @block.tensor
        def _(tensor):
            emit_scores(0)
            emit_scores(1)
            for p in range(NP):
                emit_pv(p)
                if p + 2 < NP:
                    emit_scores(p + 2)

        @block.scalar
        def _(scalar):
            for p in range(NP):
                u = p % 2
                if p >= 2:
                    scalar.wait_ge(pe_v, p - 1)  # pp[u] free: PV of p-2 done
                nc.scalar.activation(
                    pp[u][:], psS[u][:, 0:1536], EXP, scale=0.125
                ).wait_op(pe_s, p + 1, GE).then_inc(act, 1)

        @block.vector
        def _(vector):
            for p in range(NP):
                u = p % 2
                if p >= 2:
                    vector.wait_ge(st[u], 32 * (p // 2))  # ob[u]/rec[u] free
                nc.vector.reciprocal(rec[u][:, 0:1], pv[u][:, 64:65]).wait_op(
                    pe_v, p + 1, GE
                )
                nc.vector.reciprocal(rec[u][:, 1:2], pv[u][:, 192:193])
                nc.vector.drain()  # DVE pipeline RAW: rec written, read next
                nc.vector.tensor_mul(
                    ob[u][:, 0:64], pv[u][:, 0:64],
                    rec[u][:, 0:1].broadcast_to([128, 64]),
                )
                nc.vector.tensor_mul(
                    ob[u][:, 64:128], pv[u][:, 128:192],
                    rec[u][:, 1:2].broadcast_to([128, 64]),
                ).then_inc(dve, 1)

    return nc


def _prepare(inputs):
    import ml_dtypes

    bf = ml_dtypes.bfloat16
    f = np.float32
    q = np.asarray(inputs["query_layer"], f).reshape(NH, T, D)
    k = np.asarray(inputs["key_layer"], f).reshape(NH, T, D)
    v = np.asarray(inputs["value_layer"], f).reshape(NH, T, D)
    sk = np.asarray(inputs["sparse_key"], f).reshape(NH, TSP, D)
    svv = np.asarray(inputs["sparse_value"], f).reshape(NH, TSP, D)
    gk = np.asarray(inputs["global_key"], f).reshape(NH, G, D)
    gvv = np.asarray(inputs["global_value"], f).reshape(NH, G, D)
    am = np.repeat(np.asarray(inputs["attention_mask"], f)[:, 0, 0, :], H, 0)
    sm = np.repeat(np.asarray(inputs["sparse_mask"], f)[:, 0, 0, :], H, 0)
    gm = np.repeat(np.asarray(inputs["global_mask"], f)[:, 0, 0, :], H, 0)

    qt = np.ascontiguousarray(q.transpose(0, 2, 1)).astype(bf)

    lkt = np.zeros((NH, 64, LKT_W), f)
    lkt[:, :, B : B + T] = k.transpose(0, 2, 1)
    lkt = lkt.astype(bf)

    skt = np.zeros((NH, 64, SKT_W), f)
    skt[:, :, 160 : 160 + TSP] = sk.transpose(0, 2, 1)
    skt = skt.astype(bf)

    gkt = np.zeros((NH, 64, 128), f)
    gkt[:, :, :G] = gk.transpose(0, 2, 1)
    gkt = gkt.astype(bf)

    # V_aug rows scaled by exp(mask); pad rows are all-zero
    em_l = np.zeros((NH, LKT_W), f)
    em_l[:, B : B + T] = np.exp(am)
    lvp = np.zeros((NH, LKT_W, 65), f)
    lvp[:, B : B + T, :64] = v
    lvp[:, :, 64] = 1.0
    lvp *= em_l[:, :, None]
    lvp = np.ascontiguousarray(
        lvp.reshape(NH, LV_C, 128, 65).transpose(0, 2, 1, 3)
    ).reshape(NH, 128, LV_C * 65).astype(bf)

    SVP_W = 96 + SV_C * 128
    em_s = np.zeros((NH, SVP_W), f)
    em_s[:, 160 : 160 + TSP] = np.exp(sm)
    sv_pad = np.zeros((NH, SVP_W, 65), f)
    sv_pad[:, 160 : 160 + TSP, :64] = svv
    sv_pad[:, :, 64] = 1.0
    sv_pad *= em_s[:, :, None]
    svp = np.empty((NH, 4, 128, SV_C, 65), f)
    for p in range(4):
        svp[:, p] = (
            sv_pad[:, 32 * p : 32 * p + SV_C * 128]
            .reshape(NH, SV_C, 128, 65)
            .transpose(0, 2, 1, 3)
        )
    svp = np.ascontiguousarray(svp.transpose(0, 2, 1, 3, 4)).reshape(
        NH, 128, 4 * SV_C * 65
    ).astype(bf)

    gvp = np.zeros((NH, 128, 65), f)
    gvp[:, :G, :64] = gvv
    gvp[:, :G, 64] = 1.0
    gvp[:, :G] *= np.exp(gm)[:, :, None]
    gvp = gvp.astype(bf)

    return [
        {
            "qt": qt[c * SL : (c + 1) * SL],
            "lkt": lkt[c * SL : (c + 1) * SL],
            "skt": skt[c * SL : (c + 1) * SL],
            "gkt": gkt[c * SL : (c + 1) * SL],
            "lv": lvp[c * SL : (c + 1) * SL],
            "sv": svp[c * SL : (c + 1) * SL],
            "gv": gvp[c * SL : (c + 1) * SL],
        }
        for c in range(NCORES)
    ]


_NC_CACHE = {}
LAST_RESULTS = None


def kernel(**inputs):
    global LAST_RESULTS
    if "nc" not in _NC_CACHE:
        _NC_CACHE["nc"] = _build_bass()
    nc = _NC_CACHE["nc"]
    in_maps = _prepare(inputs)
    res = run_bass_kernel_spmd(nc, in_maps, core_ids=list(range(NCORES)))
    LAST_RESULTS = res
    out = np.empty((NH, T, D), np.float32)
    for c in range(NCORES):
        out[c * SL : (c + 1) * SL] = res.results[c]["o"]
    return out.reshape(N, H, T, D)
